# revision 6
# baseline (speedup 1.0000x reference)
"""Trainium2 Bass kernel for nn_Attention_59708635349115.

Decoder self-attention (GQA 16 q-heads / 4 kv-heads, RoPE, causal) over
B=2, S=2048, H=2048, distributed over 8 NeuronCores as 2 (batch) x
4 (head-group) shards.  Each core computes q/k/v projections for its
4 q-heads / 1 kv-head, causal flash-style attention, and a partial
o-projection against its 512-row slice of Wo.  The host sums the 4
partials per batch — no on-device collectives.

v2 design (vs the phase-separated v1):
  - All matmul operands are bf16 (DMA volume halved, FWL weight loads),
    accumulation stays fp32 in PSUM.  rel-err budget is 2e-2; measured
    bf16 error is ~1e-3.
  - One fused pipeline per 512-token q-chunk: projections+RoPE ->
    causal attention for the 4 heads -> partial o-projection + store.
    Engine totals (PE ~225us, ACT ~140us, DVE ~90us) overlap instead of
    serializing per phase.
  - RoPE rotate-half runs as two SBUF->SBUF partition-shift DMAs on the
    ACT HWDGE ring (sin table is sign-folded on host), freeing the PE
    matmuls + a PSUM bank that v1 spent on it.
  - Softmax denominator reciprocal via reciprocal_approx_fast (the
    exact DVE reciprocal was 3.3us per [1,512]); PV output is drained
    PSUM->SBUF by the scalar engine so the single PV PSUM bank frees
    fast.
  - PSUM budget (8 banks): proj accum 2, scores+vT-transpose 2,
    PV 1, denom 1, o-proj 2.
"""

import os
import sys

for _p in ("/opt/trn_rl_repo", "/root/.axon_site/_ro/trn_rl_repo"):
    if os.path.isdir(_p) and _p not in sys.path:
        sys.path.insert(0, _p)

import numpy as np
import ml_dtypes

import concourse.bass as bass
import concourse.mybir as mybir
import concourse.tile as tile
from concourse import bacc
from concourse.bass_utils import run_bass_kernel_spmd

B, S, H = 2, 2048, 2048
NH, NKV = 16, 4
HD = H // NH            # 128
G = 4                   # head-group shards (tensor parallel)
HPC = NH // G           # 4 q heads per core
N_CORES = 8
P = 128                 # partition dim
NQ = 512                # q-chunk (matmul moving dim)
NJ = S // NQ            # 4 q-chunks
KC = S // P             # 16 key/token 128-chunks
HC = H // P             # 16 hidden 128-chunks

F32 = mybir.dt.float32
BF = mybir.dt.bfloat16
AF = mybir.ActivationFunctionType
NPBF = ml_dtypes.bfloat16

_CACHE = {}


def _build_program(loop_n=1):
    nc = bacc.Bacc("TRN2", target_bir_lowering=False, debug=False,
                   num_devices=N_CORES)

    ext = {}
    for name, shape, dt in [
        ("xT", [H, S], BF),
        ("wq", [H, HPC * HD], BF),
        ("wk", [H, HD], BF),
        ("wv", [H, HD], BF),
        ("wo", [HPC * HD, H], BF),
        ("cosT", [HD, S], BF),
        ("sinS", [HD, S], BF),       # sign-folded: rows 0:64 negated
        ("tri", [P, P], BF),
        ("ident", [P, P], BF),
        ("mbias", [P, KC], F32),
        ("onescol", [P, 1], BF),
    ]:
        ext[name] = nc.dram_tensor(name, shape, dt, kind="ExternalInput")
    out_ext = nc.dram_tensor("out_p", [S, H], F32, kind="ExternalOutput")

    scale = float(1.0 / np.sqrt(HD))

    from contextlib import nullcontext
    with nc.allow_low_precision(reason="bf16 matmul rounding is intended"), \
         tile.TileContext(nc) as tc:
        with tc.tile_pool(name="warm", bufs=1) as warmp:
            warm_in = warmp.tile([P, 4], F32)
            warm_out = warmp.tile([P, 4], F32)
            nc.gpsimd.memset(warm_in[:], 0.0)
            nc.scalar.activation(warm_out[:], warm_in[:], AF.Exp)
        with tc.tile_pool(name="persist", bufs=1) as persist, \
             (tc.For_i(0, loop_n, 1,
                       hint_engines=(mybir.EngineType.PE,
                                     mybir.EngineType.Activation,
                                     mybir.EngineType.DVE,
                                     mybir.EngineType.Pool,
                                     mybir.EngineType.SP),
                       staggered_reset=True)
              if loop_n > 1 else nullcontext()):
            qT_all = persist.tile([P, HPC * S], BF)     # [hd, h*S + tok]
            kT_all = persist.tile([P, S], BF)
            v_all = persist.tile([P, S], BF)            # [tok%128, kc*128+hd]
            outT_all = persist.tile([P, HPC * S], BF)
            wq_sb = persist.tile([P, HC * HPC * HD], BF)
            wk_sb = persist.tile([P, HC * HD], BF)
            wv_sb = persist.tile([P, HC * HD], BF)
            wo_sb = persist.tile([P, HPC * H], BF)      # [hd, h*H + hout]
            cos_sb = persist.tile([HD, S], BF)
            sin_sb = persist.tile([HD, S], BF)
            tri_sb = persist.tile([P, P], BF)
            ident_sb = persist.tile([P, P], BF)
            mb_sb = persist.tile([P, KC], F32)
            ones_sb = persist.tile([P, 1], BF)

            with tc.tile_pool(name="xt", bufs=2) as xtp, \
                 tc.tile_pool(name="rope", bufs=3) as rope, \
                 tc.tile_pool(name="expp", bufs=6) as expp, \
                 tc.tile_pool(name="smx", bufs=2) as smx, \
                 tc.tile_pool(name="ost", bufs=2) as ost, \
                 tc.tile_pool(name="psA", bufs=2, space="PSUM") as psA, \
                 tc.tile_pool(name="psS", bufs=2, space="PSUM") as psS, \
                 tc.tile_pool(name="psO", bufs=1, space="PSUM") as psO, \
                 tc.tile_pool(name="psr", bufs=1, space="PSUM") as psr, \
                 tc.tile_pool(name="psC", bufs=2, space="PSUM") as psC:

                # ---------- input DMA, ordered by first-use time ----------
                # Batched multi-dim-AP transfers: few big dma_starts (the
                # SP sequencer pays ~600ns per issue).
                xts = {}
                QB = 4          # hidden-chunks per xt DMA batch

                def load_xt(jq):
                    xt = xtp.tile([P, HC * NQ], BF, tag="xt")
                    for c0 in range(0, HC, QB):
                        src = ext["xT"][c0 * P:(c0 + QB) * P,
                                        jq * NQ:(jq + 1) * NQ] \
                            .rearrange("(c p) q -> p c q", c=QB, p=P)
                        dst = xt[:, c0 * NQ:(c0 + QB) * NQ] \
                            .rearrange("p (c q) -> p c q", c=QB, q=NQ)
                        nc.sync.dma_start(dst, src)
                    xts[jq] = xt

                def load_cs(jq):
                    sl = slice(jq * NQ, (jq + 1) * NQ)
                    nc.sync.dma_start(cos_sb[:, sl], ext["cosT"][:, sl])
                    nc.sync.dma_start(sin_sb[:, sl], ext["sinS"][:, sl])

                load_xt(0)
                load_cs(0)
                for c0 in range(0, HC, QB):
                    src = ext["wq"][c0 * P:(c0 + QB) * P, :] \
                        .rearrange("(c p) d -> p c d", c=QB, p=P)
                    dst = wq_sb[:, c0 * HPC * HD:(c0 + QB) * HPC * HD] \
                        .rearrange("p (c d) -> p c d", c=QB, d=HPC * HD)
                    nc.sync.dma_start(dst, src)
                nc.sync.dma_start(
                    wk_sb[:].rearrange("p (c d) -> p c d", c=HC, d=HD),
                    ext["wk"][:].rearrange("(c p) d -> p c d", c=HC, p=P))
                nc.sync.dma_start(
                    wv_sb[:].rearrange("p (c d) -> p c d", c=HC, d=HD),
                    ext["wv"][:].rearrange("(c p) d -> p c d", c=HC, p=P))
                nc.sync.dma_start(tri_sb[:], ext["tri"][:])
                nc.sync.dma_start(ident_sb[:], ext["ident"][:])
                nc.sync.dma_start(mb_sb[:], ext["mbias"][:])
                nc.sync.dma_start(ones_sb[:], ext["onescol"][:])
                nc.sync.dma_start(
                    wo_sb[:].rearrange("p (h o) -> p h o", h=HPC, o=H),
                    ext["wo"][:].rearrange("(h p) o -> p h o", h=HPC, p=P))

                def rope_store(ps, dst_ap, jq):
                    """dst = raw*cos + rot64(raw)*sin_signed (bf16).

                    ACT stays exp-only: the PSUM drain runs on DVE and the
                    partition-rotate DMAs ride the SP HWDGE ring."""
                    raw = rope.tile([P, NQ], BF, tag="raw")
                    nc.vector.tensor_copy(raw[:], ps[:])
                    rot = rope.tile([P, NQ], BF, tag="rot")
                    nc.sync.dma_start(rot[0:64, :], raw[64:128, :])
                    nc.sync.dma_start(rot[64:128, :], raw[0:64, :])
                    sl = slice(jq * NQ, (jq + 1) * NQ)
                    t1 = rope.tile([P, NQ], BF, tag="t1")
                    nc.vector.tensor_mul(t1[:], raw[:], cos_sb[:, sl])
                    nc.vector.tensor_mul(rot[:], rot[:], sin_sb[:, sl])
                    nc.vector.tensor_add(dst_ap, t1[:], rot[:])

                # ================= fused per-chunk pipeline =================
                for jq in range(NJ):
                    if jq > 0 and loop_n > 1:
                        tc.stage_boundary()
                    # prefetch next chunk's activations
                    if jq + 1 < NJ:
                        load_xt(jq + 1)
                        load_cs(jq + 1)

                    # ---- projections + RoPE for this chunk ----
                    for h in range(HPC):
                        ps = psA.tile([P, NQ], F32, tag="proj")
                        for c in range(HC):
                            nc.tensor.matmul(
                                ps[:],
                                wq_sb[:, c * HPC * HD + h * HD:
                                      c * HPC * HD + (h + 1) * HD],
                                xts[jq][:, c * NQ:(c + 1) * NQ],
                                start=(c == 0), stop=(c == HC - 1))
                        rope_store(ps, qT_all[:, h * S + jq * NQ:
                                              h * S + (jq + 1) * NQ], jq)
                    ps = psA.tile([P, NQ], F32, tag="proj")
                    for c in range(HC):
                        nc.tensor.matmul(
                            ps[:], wk_sb[:, c * HD:(c + 1) * HD],
                            xts[jq][:, c * NQ:(c + 1) * NQ],
                            start=(c == 0), stop=(c == HC - 1))
                    rope_store(ps, kT_all[:, jq * NQ:(jq + 1) * NQ], jq)
                    ps = psA.tile([P, NQ], F32, tag="proj")
                    for c in range(HC):
                        nc.tensor.matmul(
                            ps[:], wv_sb[:, c * HD:(c + 1) * HD],
                            xts[jq][:, c * NQ:(c + 1) * NQ],
                            start=(c == 0), stop=(c == HC - 1))
                    vT = rope.tile([P, NQ], BF, tag="vT")
                    nc.vector.tensor_copy(vT[:], ps[:])
                    for s4 in range(NQ // P):
                        kc = jq * (NQ // P) + s4
                        pt = psS.tile([P, P], BF, tag="sc")
                        nc.tensor.transpose(
                            pt[:], vT[:, s4 * P:(s4 + 1) * P], ident_sb[:])
                        nc.vector.tensor_copy(
                            v_all[:, kc * P:(kc + 1) * P], pt[:])

                    # ---- causal attention for the 4 heads ----
                    nkc = (jq + 1) * (NQ // P)
                    for h in range(HPC):
                        q0 = h * S + jq * NQ
                        ps_out = psO.tile([P, NQ], F32, tag="pv")
                        ps_r = psr.tile([1, NQ], F32, tag="r")
                        pend = None

                        def drain(pkc, pqlo, pet):
                            nc.tensor.matmul(
                                ps_r[:, pqlo:NQ], ones_sb[:],
                                pet[:, pqlo:NQ],
                                start=(pkc == 0), stop=(pkc == nkc - 1))
                            nc.tensor.matmul(
                                ps_out[:, pqlo:NQ],
                                v_all[:, pkc * P:(pkc + 1) * P],
                                pet[:, pqlo:NQ],
                                start=(pkc == 0), stop=(pkc == nkc - 1))

                        for kc in range(nkc):
                            r = kc - jq * (NQ // P)   # straddle index
                            qlo = r * P if r >= 0 else 0
                            ps_sc = psS.tile([P, NQ], F32, tag="sc")
                            nc.tensor.matmul(
                                ps_sc[:, qlo:NQ],
                                kT_all[:, kc * P:(kc + 1) * P],
                                qT_all[:, q0 + qlo:q0 + NQ],
                                start=True, stop=True)
                            et = expp.tile([P, NQ], BF, tag="exp")
                            nc.scalar.activation(
                                et[:, qlo:NQ], ps_sc[:, qlo:NQ], AF.Exp,
                                scale=scale, bias=mb_sb[:, kc:kc + 1])
                            if r >= 0:
                                nc.vector.tensor_mul(
                                    et[:, qlo:qlo + P], et[:, qlo:qlo + P],
                                    tri_sb[:])
                            if pend is not None:
                                drain(*pend)
                            pend = (kc, qlo, et)
                        drain(*pend)
                        # normalize: drain PV fast, 1/denom, broadcast, mul
                        unorm = smx.tile([P, NQ], F32, tag="unorm")
                        nc.vector.tensor_copy(unorm[:], ps_out[:])
                        recip = smx.tile([1, NQ], F32, tag="recip")
                        nc.vector.reciprocal_approx_fast(recip[:], ps_r[:])
                        bcast = smx.tile([P, NQ], F32, tag="bcast")
                        nc.gpsimd.partition_broadcast(bcast[:], recip[:])
                        nc.vector.tensor_mul(
                            outT_all[:, q0:q0 + NQ], unorm[:], bcast[:])

                    # ---- partial o-projection for this chunk's tokens ----
                    for tc_i in range(jq * (NQ // P), (jq + 1) * (NQ // P)):
                        st = ost.tile([P, H], F32, tag="st")
                        for n in range(H // NQ):
                            ps = psC.tile([P, NQ], F32, tag="op")
                            for h in range(HPC):
                                nc.tensor.matmul(
                                    ps[:],
                                    outT_all[:, h * S + tc_i * P:
                                             h * S + (tc_i + 1) * P],
                                    wo_sb[:, h * H + n * NQ:
                                          h * H + (n + 1) * NQ],
                                    start=(h == 0), stop=(h == HPC - 1))
                            nc.vector.tensor_copy(
                                st[:, n * NQ:(n + 1) * NQ], ps[:])
                        nc.sync.dma_start(
                            out_ext[tc_i * P:(tc_i + 1) * P, :], st[:])

    nc.compile()
    return nc


def _host_consts():
    tri = np.triu(np.ones((P, P), dtype=np.float32))   # keep k_local <= q_local
    ident = np.eye(P, dtype=np.float32)
    onescol = np.ones((P, 1), dtype=np.float32)
    return tri, ident, onescol


def build_in_maps(hidden_states, cos, sin, Wq, Wk, Wv, Wo, attention_mask):
    tri, ident, onescol = _host_consts()
    cosT = np.ascontiguousarray(cos.T.astype(NPBF))
    # sign-folded sin for DMA-based rotate-half: rows 0:64 get -sin
    sinT = sin.T.astype(np.float32).copy()
    sinT[:HD // 2, :] *= -1.0
    sinS = np.ascontiguousarray(sinT.astype(NPBF))
    in_maps = []
    for core in range(N_CORES):
        b, g = divmod(core, G)
        xT = np.ascontiguousarray(hidden_states[b].T.astype(NPBF))
        mb = ((attention_mask[b].astype(np.float32) - 1.0) * 1e30)
        mb = np.ascontiguousarray(mb.reshape(KC, P).T)
        in_maps.append({
            "xT": xT,
            "wq": np.ascontiguousarray(
                Wq[:, g * HPC * HD:(g + 1) * HPC * HD].astype(NPBF)),
            "wk": np.ascontiguousarray(Wk[:, g * HD:(g + 1) * HD].astype(NPBF)),
            "wv": np.ascontiguousarray(Wv[:, g * HD:(g + 1) * HD].astype(NPBF)),
            "wo": np.ascontiguousarray(
                Wo[g * HPC * HD:(g + 1) * HPC * HD, :].astype(NPBF)),
            "cosT": cosT, "sinS": sinS,
            "tri": tri.astype(NPBF), "ident": ident.astype(NPBF),
            "mbias": mb, "onescol": onescol.astype(NPBF),
        })
    return in_maps


def kernel(hidden_states, cos, sin, Wq, Wk, Wv, Wo, attention_mask):
    if "nc" not in _CACHE:
        _CACHE["nc"] = _build_program()
    nc = _CACHE["nc"]
    in_maps = build_in_maps(np.asarray(hidden_states, np.float32),
                            np.asarray(cos, np.float32),
                            np.asarray(sin, np.float32),
                            np.asarray(Wq, np.float32),
                            np.asarray(Wk, np.float32),
                            np.asarray(Wv, np.float32),
                            np.asarray(Wo, np.float32),
                            np.asarray(attention_mask, np.float32))
    res = run_bass_kernel_spmd(nc, in_maps, list(range(N_CORES)))
    out = np.empty((B, S, H), dtype=np.float32)
    for b in range(B):
        acc = res.results[4 * b]["out_p"].astype(np.float32)
        for g in range(1, G):
            acc = acc + res.results[4 * b + g]["out_p"]
        out[b] = acc
    return out


if __name__ == "__main__":
    rng = np.random.default_rng(0)
    hs = rng.standard_normal((B, S, H), dtype=np.float32)
    inv_freq = 1.0 / (10000.0 ** (np.arange(0, HD, 2, dtype=np.float32) / HD))
    t = np.arange(S, dtype=np.float32)
    freqs = np.outer(t, inv_freq)
    emb = np.concatenate([freqs, freqs], axis=-1)
    out = kernel(hs, np.cos(emb), np.sin(emb),
                 rng.standard_normal((H, NH * HD), dtype=np.float32) * 0.02,
                 rng.standard_normal((H, NKV * HD), dtype=np.float32) * 0.02,
                 rng.standard_normal((H, NKV * HD), dtype=np.float32) * 0.02,
                 rng.standard_normal((NH * HD, H), dtype=np.float32) * 0.02,
                 np.ones((B, S), dtype=np.float32))
    print("kernel ran, out shape", out.shape, "finite:", np.isfinite(out).all())


# revision 7
# speedup vs baseline: 1.1950x; 1.1950x over previous
"""Trainium2 Bass kernel for nn_Attention_59708635349115.

Decoder self-attention (GQA 16 q-heads / 4 kv-heads, RoPE, causal) over
B=2, S=2048, H=2048, distributed over 8 NeuronCores as 2 (batch) x
4 (head-group) shards.  Each core computes q/k/v projections for its
4 q-heads / 1 kv-head, causal flash-style attention, and a partial
o-projection against its 512-row slice of Wo.  The host sums the 4
partials per batch — no on-device collectives.

v2 design (vs the phase-separated v1):
  - All matmul operands are bf16 (DMA volume halved, FWL weight loads),
    accumulation stays fp32 in PSUM.  rel-err budget is 2e-2; measured
    bf16 error is ~1e-3.
  - One fused pipeline per 512-token q-chunk: projections+RoPE ->
    causal attention for the 4 heads -> partial o-projection + store.
    Engine totals (PE ~225us, ACT ~140us, DVE ~90us) overlap instead of
    serializing per phase.
  - RoPE rotate-half runs as two SBUF->SBUF partition-shift DMAs on the
    ACT HWDGE ring (sin table is sign-folded on host), freeing the PE
    matmuls + a PSUM bank that v1 spent on it.
  - Softmax denominator reciprocal via reciprocal_approx_fast (the
    exact DVE reciprocal was 3.3us per [1,512]); PV output is drained
    PSUM->SBUF by the scalar engine so the single PV PSUM bank frees
    fast.
  - PSUM budget (8 banks): proj accum 2, scores+vT-transpose 2,
    PV 1, denom 1, o-proj 2.
"""

import os
import sys

for _p in ("/opt/trn_rl_repo", "/root/.axon_site/_ro/trn_rl_repo"):
    if os.path.isdir(_p) and _p not in sys.path:
        sys.path.insert(0, _p)

import numpy as np
import ml_dtypes

import concourse.bass as bass
import concourse.mybir as mybir
import concourse.tile as tile
from concourse import bacc
from concourse.bass_utils import run_bass_kernel_spmd

B, S, H = 2, 2048, 2048
NH, NKV = 16, 4
HD = H // NH            # 128
G = 4                   # head-group shards (tensor parallel)
HPC = NH // G           # 4 q heads per core
N_CORES = 8
P = 128                 # partition dim
NQ = 512                # q-chunk (matmul moving dim)
NJ = S // NQ            # 4 q-chunks
KC = S // P             # 16 key/token 128-chunks
HC = H // P             # 16 hidden 128-chunks

F32 = mybir.dt.float32
BF = mybir.dt.bfloat16
AF = mybir.ActivationFunctionType
NPBF = ml_dtypes.bfloat16

_CACHE = {}


def _build_program(loop_n=1):
    nc = bacc.Bacc("TRN2", target_bir_lowering=False, debug=False,
                   num_devices=N_CORES)

    ext = {}
    for name, shape, dt in [
        ("xT", [H, S], BF),
        ("wq", [H, HPC * HD], BF),
        ("wk", [H, HD], BF),
        ("wv", [H, HD], BF),
        ("wo", [HPC * HD, H], BF),
        ("cosT", [HD, S], BF),
        ("sinS", [HD, S], BF),       # sign-folded: rows 0:64 negated
        ("tri", [P, P], BF),
        ("ident", [P, P], BF),
        ("mbias", [P, KC], F32),
        ("onescol", [P, 1], BF),
    ]:
        ext[name] = nc.dram_tensor(name, shape, dt, kind="ExternalInput")
    out_ext = nc.dram_tensor("out_p", [S, H], F32, kind="ExternalOutput")

    scale = float(1.0 / np.sqrt(HD))

    from contextlib import nullcontext
    with nc.allow_low_precision(reason="bf16 matmul rounding is intended"), \
         tile.TileContext(nc) as tc:
        with tc.tile_pool(name="warm", bufs=1) as warmp:
            warm_in = warmp.tile([P, 4], F32)
            warm_out = warmp.tile([P, 4], F32)
            nc.gpsimd.memset(warm_in[:], 0.0)
            nc.scalar.activation(warm_out[:], warm_in[:], AF.Exp)
        with tc.tile_pool(name="persist", bufs=1) as persist, \
             (tc.For_i(0, loop_n, 1,
                       hint_engines=(mybir.EngineType.PE,
                                     mybir.EngineType.Activation,
                                     mybir.EngineType.DVE,
                                     mybir.EngineType.Pool,
                                     mybir.EngineType.SP))
              if loop_n > 1 else nullcontext()):
            qT_all = persist.tile([P, HPC * S], BF)     # [hd, h*S + tok]
            kT_all = persist.tile([P, S], BF)
            v_all = persist.tile([P, S], BF)            # [tok%128, kc*128+hd]
            outT_all = persist.tile([P, HPC * S], BF)
            wq_sb = persist.tile([P, HC * HPC * HD], BF)
            wk_sb = persist.tile([P, HC * HD], BF)
            wv_sb = persist.tile([P, HC * HD], BF)
            wo_sb = persist.tile([P, HPC * H], BF)      # [hd, h*H + hout]
            cos_sb = persist.tile([HD, S], BF)
            sin_sb = persist.tile([HD, S], BF)
            tri_sb = persist.tile([P, P], BF)
            ident_sb = persist.tile([P, P], BF)
            mb_sb = persist.tile([P, KC], F32)
            ones_sb = persist.tile([P, 1], BF)

            with tc.tile_pool(name="xt", bufs=2) as xtp, \
                 tc.tile_pool(name="rope", bufs=3) as rope, \
                 tc.tile_pool(name="expp", bufs=6) as expp, \
                 tc.tile_pool(name="smx", bufs=2) as smx, \
                 tc.tile_pool(name="ost", bufs=2) as ost, \
                 tc.tile_pool(name="psA", bufs=2, space="PSUM") as psA, \
                 tc.tile_pool(name="psS", bufs=2, space="PSUM") as psS, \
                 tc.tile_pool(name="psO", bufs=1, space="PSUM") as psO, \
                 tc.tile_pool(name="psr", bufs=1, space="PSUM") as psr, \
                 tc.tile_pool(name="psC", bufs=2, space="PSUM") as psC:

                # ---------- input DMA, ordered by first-use time ----------
                # Batched multi-dim-AP transfers: few big dma_starts (the
                # SP sequencer pays ~600ns per issue).
                xts = {}
                QB = 4          # hidden-chunks per xt DMA batch

                def load_xt(jq):
                    xt = xtp.tile([P, HC * NQ], BF, tag="xt")
                    for c0 in range(0, HC, QB):
                        src = ext["xT"][c0 * P:(c0 + QB) * P,
                                        jq * NQ:(jq + 1) * NQ] \
                            .rearrange("(c p) q -> p c q", c=QB, p=P)
                        dst = xt[:, c0 * NQ:(c0 + QB) * NQ] \
                            .rearrange("p (c q) -> p c q", c=QB, q=NQ)
                        nc.sync.dma_start(dst, src)
                    xts[jq] = xt

                def load_cs(jq):
                    sl = slice(jq * NQ, (jq + 1) * NQ)
                    nc.sync.dma_start(cos_sb[:, sl], ext["cosT"][:, sl])
                    nc.sync.dma_start(sin_sb[:, sl], ext["sinS"][:, sl])

                load_xt(0)
                load_cs(0)
                for c0 in range(0, HC, QB):
                    src = ext["wq"][c0 * P:(c0 + QB) * P, :] \
                        .rearrange("(c p) d -> p c d", c=QB, p=P)
                    dst = wq_sb[:, c0 * HPC * HD:(c0 + QB) * HPC * HD] \
                        .rearrange("p (c d) -> p c d", c=QB, d=HPC * HD)
                    nc.sync.dma_start(dst, src)
                nc.sync.dma_start(
                    wk_sb[:].rearrange("p (c d) -> p c d", c=HC, d=HD),
                    ext["wk"][:].rearrange("(c p) d -> p c d", c=HC, p=P))
                nc.sync.dma_start(
                    wv_sb[:].rearrange("p (c d) -> p c d", c=HC, d=HD),
                    ext["wv"][:].rearrange("(c p) d -> p c d", c=HC, p=P))
                nc.sync.dma_start(tri_sb[:], ext["tri"][:])
                nc.sync.dma_start(ident_sb[:], ext["ident"][:])
                nc.sync.dma_start(mb_sb[:], ext["mbias"][:])
                nc.sync.dma_start(ones_sb[:], ext["onescol"][:])
                nc.sync.dma_start(
                    wo_sb[:].rearrange("p (h o) -> p h o", h=HPC, o=H),
                    ext["wo"][:].rearrange("(h p) o -> p h o", h=HPC, p=P))

                def rope_store(ps, dst_ap, jq):
                    """dst = raw*cos + rot64(raw)*sin_signed (bf16).

                    ACT stays exp-only: the PSUM drain runs on DVE and the
                    partition-rotate DMAs ride the SP HWDGE ring."""
                    raw = rope.tile([P, NQ], BF, tag="raw")
                    nc.vector.tensor_copy(raw[:], ps[:])
                    rot = rope.tile([P, NQ], BF, tag="rot")
                    nc.sync.dma_start(rot[0:64, :], raw[64:128, :])
                    nc.sync.dma_start(rot[64:128, :], raw[0:64, :])
                    sl = slice(jq * NQ, (jq + 1) * NQ)
                    t1 = rope.tile([P, NQ], BF, tag="t1")
                    nc.vector.tensor_mul(t1[:], raw[:], cos_sb[:, sl])
                    nc.vector.tensor_mul(rot[:], rot[:], sin_sb[:, sl])
                    nc.vector.tensor_add(dst_ap, t1[:], rot[:])

                # ================= fused per-chunk pipeline =================
                for jq in range(NJ):
                    # prefetch next chunk's activations
                    if jq + 1 < NJ:
                        load_xt(jq + 1)
                        load_cs(jq + 1)

                    # ---- projections + RoPE for this chunk ----
                    for h in range(HPC):
                        ps = psA.tile([P, NQ], F32, tag="proj")
                        for c in range(HC):
                            nc.tensor.matmul(
                                ps[:],
                                wq_sb[:, c * HPC * HD + h * HD:
                                      c * HPC * HD + (h + 1) * HD],
                                xts[jq][:, c * NQ:(c + 1) * NQ],
                                start=(c == 0), stop=(c == HC - 1))
                        rope_store(ps, qT_all[:, h * S + jq * NQ:
                                              h * S + (jq + 1) * NQ], jq)
                    ps = psA.tile([P, NQ], F32, tag="proj")
                    for c in range(HC):
                        nc.tensor.matmul(
                            ps[:], wk_sb[:, c * HD:(c + 1) * HD],
                            xts[jq][:, c * NQ:(c + 1) * NQ],
                            start=(c == 0), stop=(c == HC - 1))
                    rope_store(ps, kT_all[:, jq * NQ:(jq + 1) * NQ], jq)
                    ps = psA.tile([P, NQ], F32, tag="proj")
                    for c in range(HC):
                        nc.tensor.matmul(
                            ps[:], wv_sb[:, c * HD:(c + 1) * HD],
                            xts[jq][:, c * NQ:(c + 1) * NQ],
                            start=(c == 0), stop=(c == HC - 1))
                    vT = rope.tile([P, NQ], BF, tag="vT")
                    nc.vector.tensor_copy(vT[:], ps[:])
                    for s4 in range(NQ // P):
                        kc = jq * (NQ // P) + s4
                        pt = psS.tile([P, P], BF, tag="sc")
                        nc.tensor.transpose(
                            pt[:], vT[:, s4 * P:(s4 + 1) * P], ident_sb[:])
                        nc.vector.tensor_copy(
                            v_all[:, kc * P:(kc + 1) * P], pt[:])

                    # ---- causal attention for the 4 heads ----
                    nkc = (jq + 1) * (NQ // P)
                    for h in range(HPC):
                        q0 = h * S + jq * NQ
                        ps_out = psO.tile([P, NQ], F32, tag="pv")
                        ps_r = psr.tile([1, NQ], F32, tag="r")
                        pend = None

                        def drain(pkc, pqlo, pet):
                            nc.tensor.matmul(
                                ps_r[:, pqlo:NQ], ones_sb[:],
                                pet[:, pqlo:NQ],
                                start=(pkc == 0), stop=(pkc == nkc - 1))
                            nc.tensor.matmul(
                                ps_out[:, pqlo:NQ],
                                v_all[:, pkc * P:(pkc + 1) * P],
                                pet[:, pqlo:NQ],
                                start=(pkc == 0), stop=(pkc == nkc - 1))

                        for kc in range(nkc):
                            r = kc - jq * (NQ // P)   # straddle index
                            qlo = r * P if r >= 0 else 0
                            ps_sc = psS.tile([P, NQ], F32, tag="sc")
                            nc.tensor.matmul(
                                ps_sc[:, qlo:NQ],
                                kT_all[:, kc * P:(kc + 1) * P],
                                qT_all[:, q0 + qlo:q0 + NQ],
                                start=True, stop=True)
                            et = expp.tile([P, NQ], BF, tag="exp")
                            nc.scalar.activation(
                                et[:, qlo:NQ], ps_sc[:, qlo:NQ], AF.Exp,
                                scale=scale, bias=mb_sb[:, kc:kc + 1])
                            if r >= 0:
                                nc.vector.tensor_mul(
                                    et[:, qlo:qlo + P], et[:, qlo:qlo + P],
                                    tri_sb[:])
                            if pend is not None:
                                drain(*pend)
                            pend = (kc, qlo, et)
                        drain(*pend)
                        # normalize: drain PV fast, 1/denom, broadcast, mul
                        unorm = smx.tile([P, NQ], F32, tag="unorm")
                        nc.vector.tensor_copy(unorm[:], ps_out[:])
                        recip = smx.tile([1, NQ], F32, tag="recip")
                        nc.vector.reciprocal_approx_fast(recip[:], ps_r[:])
                        bcast = smx.tile([P, NQ], F32, tag="bcast")
                        nc.gpsimd.partition_broadcast(bcast[:], recip[:])
                        nc.vector.tensor_mul(
                            outT_all[:, q0:q0 + NQ], unorm[:], bcast[:])

                    # ---- partial o-projection for this chunk's tokens ----
                    for tc_i in range(jq * (NQ // P), (jq + 1) * (NQ // P)):
                        st = ost.tile([P, H], F32, tag="st")
                        for n in range(H // NQ):
                            ps = psC.tile([P, NQ], F32, tag="op")
                            for h in range(HPC):
                                nc.tensor.matmul(
                                    ps[:],
                                    outT_all[:, h * S + tc_i * P:
                                             h * S + (tc_i + 1) * P],
                                    wo_sb[:, h * H + n * NQ:
                                          h * H + (n + 1) * NQ],
                                    start=(h == 0), stop=(h == HPC - 1))
                            nc.vector.tensor_copy(
                                st[:, n * NQ:(n + 1) * NQ], ps[:])
                        nc.sync.dma_start(
                            out_ext[tc_i * P:(tc_i + 1) * P, :], st[:])

    nc.compile()
    return nc


def _host_consts():
    tri = np.triu(np.ones((P, P), dtype=np.float32))   # keep k_local <= q_local
    ident = np.eye(P, dtype=np.float32)
    onescol = np.ones((P, 1), dtype=np.float32)
    return tri, ident, onescol


def build_in_maps(hidden_states, cos, sin, Wq, Wk, Wv, Wo, attention_mask):
    tri, ident, onescol = _host_consts()
    cosT = np.ascontiguousarray(cos.T.astype(NPBF))
    # sign-folded sin for DMA-based rotate-half: rows 0:64 get -sin
    sinT = sin.T.astype(np.float32).copy()
    sinT[:HD // 2, :] *= -1.0
    sinS = np.ascontiguousarray(sinT.astype(NPBF))
    in_maps = []
    for core in range(N_CORES):
        b, g = divmod(core, G)
        xT = np.ascontiguousarray(hidden_states[b].T.astype(NPBF))
        mb = ((attention_mask[b].astype(np.float32) - 1.0) * 1e30)
        mb = np.ascontiguousarray(mb.reshape(KC, P).T)
        in_maps.append({
            "xT": xT,
            "wq": np.ascontiguousarray(
                Wq[:, g * HPC * HD:(g + 1) * HPC * HD].astype(NPBF)),
            "wk": np.ascontiguousarray(Wk[:, g * HD:(g + 1) * HD].astype(NPBF)),
            "wv": np.ascontiguousarray(Wv[:, g * HD:(g + 1) * HD].astype(NPBF)),
            "wo": np.ascontiguousarray(
                Wo[g * HPC * HD:(g + 1) * HPC * HD, :].astype(NPBF)),
            "cosT": cosT, "sinS": sinS,
            "tri": tri.astype(NPBF), "ident": ident.astype(NPBF),
            "mbias": mb, "onescol": onescol.astype(NPBF),
        })
    return in_maps


def kernel(hidden_states, cos, sin, Wq, Wk, Wv, Wo, attention_mask):
    if "nc" not in _CACHE:
        _CACHE["nc"] = _build_program()
    nc = _CACHE["nc"]
    in_maps = build_in_maps(np.asarray(hidden_states, np.float32),
                            np.asarray(cos, np.float32),
                            np.asarray(sin, np.float32),
                            np.asarray(Wq, np.float32),
                            np.asarray(Wk, np.float32),
                            np.asarray(Wv, np.float32),
                            np.asarray(Wo, np.float32),
                            np.asarray(attention_mask, np.float32))
    res = run_bass_kernel_spmd(nc, in_maps, list(range(N_CORES)))
    out = np.empty((B, S, H), dtype=np.float32)
    for b in range(B):
        acc = res.results[4 * b]["out_p"].astype(np.float32)
        for g in range(1, G):
            acc = acc + res.results[4 * b + g]["out_p"]
        out[b] = acc
    return out


if __name__ == "__main__":
    rng = np.random.default_rng(0)
    hs = rng.standard_normal((B, S, H), dtype=np.float32)
    inv_freq = 1.0 / (10000.0 ** (np.arange(0, HD, 2, dtype=np.float32) / HD))
    t = np.arange(S, dtype=np.float32)
    freqs = np.outer(t, inv_freq)
    emb = np.concatenate([freqs, freqs], axis=-1)
    out = kernel(hs, np.cos(emb), np.sin(emb),
                 rng.standard_normal((H, NH * HD), dtype=np.float32) * 0.02,
                 rng.standard_normal((H, NKV * HD), dtype=np.float32) * 0.02,
                 rng.standard_normal((H, NKV * HD), dtype=np.float32) * 0.02,
                 rng.standard_normal((NH * HD, H), dtype=np.float32) * 0.02,
                 np.ones((B, S), dtype=np.float32))
    print("kernel ran, out shape", out.shape, "finite:", np.isfinite(out).all())
